# revision 9
# baseline (speedup 1.0000x reference)
"""ConvLSTM Trainium2 kernel (Bass/Tile), data-parallel over batch on 8 cores.

Problem shapes (hardcoded): x[16,64,64,256] f32, W_conv[512,192,5], b_conv[512].
reference: per step t, conv1d(concat(x_t, h), W_conv) 'same' pad -> 4 gates ->
LSTM cell update. Returns (outputs[B,T,128,256], h_last, c_last).

Per-core mapping (2 samples/core):
  * conv expressed as PSUM-accumulated matmuls: contraction 960 = 192ch x 5taps,
    done as 8 matmuls of contract-128 per 128-wide output chunk (4 chunks),
    free dim N = 2 samples x 256 = 512 (one full PSUM bank).
  * h-part (128 ch): 5 tap-shifted matmuls against a zero-padded persistent
    h buffer [128, 2, 260]; tap shift = free-dim AP offset (free).
  * x-part (64 ch): two taps packed per matmul: x stored twice in one tile
    (partitions 0-63 unshifted, 64-127 shifted by one column) -> 3 matmuls,
    with tap-paired weight layout prepared host-side.
  * fp32r matmuls (full PE rate at free dim >= 256), fp32 PSUM accumulate.
  * gates: ScalarE activation with fused per-partition bias (sigmoid/tanh are
    in one ACT table set); cell update on VectorE.
"""

import sys

import numpy as np

for _p in ("/opt/trn_rl_repo", "/root/.axon_site/_ro/trn_rl_repo"):
    if _p not in sys.path:
        sys.path.append(_p)

HIDDEN = 128
B, T, C, W = 16, 64, 64, 256
KTAPS = 5
PAD = KTAPS // 2           # 2
WP = W + 2 * PAD           # 260
NCORES = 8
BPC = B // NCORES          # 2 samples per core
NM = 4                     # output-channel chunks of 128 (i, f, o, g)

TRACE = False              # test.py sets True to capture NTFF profile
TRACE_KWARGS = {}
LAST_RESULT = None         # BassKernelResults of the last run (for timing)

_BUILT = {}


def _build(t_steps=T):
    """Build + compile the single-core Bass program (SPMD across cores)."""
    import concourse.bacc as bacc
    import concourse.mybir as mybir
    import concourse.tile as tile

    f32 = mybir.dt.float32
    f32r = mybir.dt.float32r
    AF = mybir.ActivationFunctionType

    nc = bacc.Bacc("TRN2", target_bir_lowering=False, debug=False)

    # matmul operands are float32r (fp32 RNE-rounded to 11 mantissa bits);
    # x/weights are pre-rounded host-side so their DMAs are plain byte copies.
    xh_d = nc.dram_tensor("xh", [t_steps, C, BPC, W], f32r, kind="ExternalInput")
    wh_d = nc.dram_tensor("wh", [NM, KTAPS, HIDDEN, 128], f32r, kind="ExternalInput")
    wx_d = nc.dram_tensor("wx", [NM, 3, 128, 128], f32r, kind="ExternalInput")
    b_d = nc.dram_tensor("bias", [NM, 128, 1], f32, kind="ExternalInput")
    out_d = nc.dram_tensor("out", [t_steps, HIDDEN, BPC, W], f32r, kind="ExternalOutput")
    cout_d = nc.dram_tensor("c_out", [HIDDEN, BPC, W], f32, kind="ExternalOutput")

    xh, wh, wx, bias, out, cout = (
        h.ap() for h in (xh_d, wh_d, wx_d, b_d, out_d, cout_d)
    )

    # matmul emission order of gate chunks: o (chunk 2) last so the
    # sig(o)*tanh(c) tail starts as early as possible.
    MORDER = [0, 1, 3, 2]

    with tile.TileContext(nc) as tc:
        with tc.tile_pool(name="wpool", bufs=1) as wpool, \
             tc.tile_pool(name="spool", bufs=1) as spool, \
             tc.tile_pool(name="gpool", bufs=3) as gpool, \
             tc.tile_pool(name="pspool", bufs=8, space="PSUM") as pspool:

            # --- weights + bias (resident for the whole kernel) ---
            wh_t = [
                [wpool.tile([HIDDEN, 128], f32r, name=f"wh_{m}_{k}")
                 for k in range(KTAPS)]
                for m in range(NM)
            ]
            wx_t = [
                [wpool.tile([128, 128], f32r, name=f"wx_{m}_{p}") for p in range(3)]
                for m in range(NM)
            ]
            b_t = [wpool.tile([128, 1], f32, name=f"b_{m}") for m in range(NM)]
            for m in range(NM):
                for k in range(KTAPS):
                    nc.sync.dma_start(out=wh_t[m][k], in_=wh[m, k])
                for p in range(3):
                    nc.sync.dma_start(out=wx_t[m][p], in_=wx[m, p])
                nc.sync.dma_start(out=b_t[m], in_=bias[m])

            # --- persistent state ---
            NXBUF = 3
            x2 = [spool.tile([128, BPC, WP], f32r, name=f"x2_{i}") for i in range(NXBUF)]
            hp = [spool.tile([HIDDEN, BPC, WP], f32r, name=f"hp_{i}") for i in range(2)]
            c_st = spool.tile([HIDDEN, BPC, W], f32, name="c_st")
            for buf in (*x2, *hp):
                nc.gpsimd.memset(buf.bitcast(mybir.dt.uint32), 0)
            nc.gpsimd.memset(c_st, 0.0)

            for t in range(t_steps):
                xt = x2[t % NXBUF]
                # top half: x padded by 2; bottom half: same shifted left by 1
                nc.sync.dma_start(out=xt[0:C, :, PAD:PAD + W], in_=xh[t])
                nc.sync.dma_start(out=xt[C:2 * C, :, PAD - 1:PAD - 1 + W], in_=xh[t])
                h_in = hp[t % 2]
                h_out = hp[(t + 1) % 2]

                ps = {
                    m: pspool.tile([128, BPC, W], f32, tag="ps", name=f"ps_{t}_{m}")
                    for m in MORDER
                }
                # x-part first: independent of h, keeps PE busy while the
                # previous step's elementwise tail computes h.
                for m in MORDER:
                    for p in range(3):
                        nc.tensor.matmul(
                            ps[m],
                            wx_t[m][p],
                            xt[:, :, 2 * p:2 * p + W],
                            start=(p == 0),
                            stop=False,
                        )
                for m in MORDER:
                    for k in range(KTAPS):
                        nc.tensor.matmul(
                            ps[m],
                            wh_t[m][k],
                            h_in[:, :, k:k + W],
                            start=False,
                            stop=(k == KTAPS - 1),
                        )

                sig_i = gpool.tile([128, BPC, W], f32, tag="si", name=f"si_{t}")
                sig_f = gpool.tile([128, BPC, W], f32, tag="sf", name=f"sf_{t}")
                tanh_g = gpool.tile([128, BPC, W], f32, tag="tg", name=f"tg_{t}")
                sig_o = gpool.tile([128, BPC, W], f32, tag="so", name=f"so_{t}")
                nc.scalar.activation(out=sig_i, in_=ps[0], func=AF.Sigmoid, bias=b_t[0])
                nc.scalar.activation(out=sig_f, in_=ps[1], func=AF.Sigmoid, bias=b_t[1])
                nc.scalar.activation(out=tanh_g, in_=ps[3], func=AF.Tanh, bias=b_t[3])
                nc.scalar.activation(out=sig_o, in_=ps[2], func=AF.Sigmoid, bias=b_t[2])

                ig = gpool.tile([128, BPC, W], f32, tag="ig", name=f"ig_{t}")
                fc = gpool.tile([128, BPC, W], f32, tag="fc", name=f"fc_{t}")
                nc.vector.tensor_mul(ig, sig_i, tanh_g)
                nc.vector.tensor_mul(fc, sig_f, c_st)
                nc.vector.tensor_add(c_st, ig, fc)
                tnc = gpool.tile([128, BPC, W], f32, tag="tnc", name=f"tnc_{t}")
                nc.scalar.activation(out=tnc, in_=c_st, func=AF.Tanh)
                nc.vector.tensor_mul(h_out[:, :, PAD:PAD + W], sig_o, tnc)
                nc.sync.dma_start(out=out[t], in_=h_out[:, :, PAD:PAD + W])

            nc.sync.dma_start(out=cout, in_=c_st)

    nc.compile()
    return nc


def get_built(t_steps=T):
    if t_steps not in _BUILT:
        _BUILT[t_steps] = _build(t_steps)
    return _BUILT[t_steps]


def round_fp32r(a):
    """RNE-round fp32 to the fp32r grid (11 mantissa bits, low 12 bits zero)."""
    u = np.ascontiguousarray(a, np.float32).view(np.uint32)
    u = (u + np.uint32(0x7FF) + ((u >> np.uint32(12)) & np.uint32(1))) \
        & np.uint32(0xFFFFF000)
    return u.view(np.float32)


def prep_weights(W_conv, b_conv):
    """Host-side weight layout: transposed lhsT tiles + tap-paired x weights."""
    Wc = round_fp32r(np.asarray(W_conv, dtype=np.float32))
    bc = np.asarray(b_conv, dtype=np.float32)
    W4 = Wc.reshape(NM, 128, C + HIDDEN, KTAPS)            # [m, co, ci, k]
    wh = np.ascontiguousarray(W4[:, :, C:, :].transpose(0, 3, 2, 1))  # [m,k,ci,co]
    Wx4 = W4[:, :, :C, :]                                   # [m, co, 64, 5]
    wx = np.zeros((NM, 3, 128, 128), np.float32)
    for p in range(2):
        wx[:, p, 0:C, :] = Wx4[:, :, :, 2 * p].transpose(0, 2, 1)
        wx[:, p, C:2 * C, :] = Wx4[:, :, :, 2 * p + 1].transpose(0, 2, 1)
    wx[:, 2, 0:C, :] = Wx4[:, :, :, 4].transpose(0, 2, 1)
    bias = np.ascontiguousarray(bc.reshape(NM, 128, 1))
    return wh, wx, bias


def make_in_maps(x, W_conv, b_conv, t_steps=T):
    x = round_fp32r(np.asarray(x, dtype=np.float32))
    wh, wx, bias = prep_weights(W_conv, b_conv)
    in_maps = []
    for cid in range(NCORES):
        xs = x[cid * BPC:(cid + 1) * BPC, :t_steps]         # [2, T', C, W]
        xh = np.ascontiguousarray(xs.transpose(1, 2, 0, 3))  # [T', C, 2, W]
        in_maps.append({"xh": xh, "wh": wh, "wx": wx, "bias": bias})
    return in_maps


def assemble(results, t_steps=T):
    outputs = np.empty((B, t_steps, HIDDEN, W), np.float32)
    c_full = np.empty((B, HIDDEN, W), np.float32)
    for cid in range(NCORES):
        o = results[cid]["out"]                              # [T', 128, 2, W]
        outputs[cid * BPC:(cid + 1) * BPC] = o.transpose(2, 0, 1, 3)
        c_full[cid * BPC:(cid + 1) * BPC] = results[cid]["c_out"].transpose(1, 0, 2)
    h_full = outputs[:, -1].copy()
    return outputs, h_full, c_full


def _install_ntff_shim():
    """The image's antenv package lacks axon_hooks; recreate it so
    run_bass_kernel_spmd(trace=True) can capture NTFF profiles."""
    import types

    if "antenv.axon_hooks" in sys.modules:
        return
    try:
        import antenv
        from trn_agent_boot.trn_boot import _ntff_profile_via_ctypes
    except ImportError:
        return
    hooks = types.ModuleType("antenv.axon_hooks")
    hooks._hook = _ntff_profile_via_ctypes("/opt/axon/libaxon_pjrt.so")
    hooks.get_axon_ntff_profile_hook = lambda: hooks._hook
    hooks.set_axon_ntff_profile_hook = lambda h: setattr(hooks, "_hook", h)
    sys.modules["antenv.axon_hooks"] = hooks
    antenv.axon_hooks = hooks


def kernel(x, W_conv, b_conv):
    global LAST_RESULT
    from concourse import bass_utils

    if TRACE:
        _install_ntff_shim()

    nc = get_built(T)
    in_maps = make_in_maps(x, W_conv, b_conv, T)
    res = bass_utils.run_bass_kernel_spmd(
        nc, in_maps, core_ids=list(range(NCORES)), trace=TRACE, **TRACE_KWARGS
    )
    LAST_RESULT = res
    return assemble(res.results, T)


# revision 12
# speedup vs baseline: 1.0304x; 1.0304x over previous
"""ConvLSTM Trainium2 kernel (Bass/Tile), data-parallel over batch on 8 cores.

Problem shapes (hardcoded): x[16,64,64,256] f32, W_conv[512,192,5], b_conv[512].
reference: per step t, conv1d(concat(x_t, h), W_conv) 'same' pad -> 4 gates ->
LSTM cell update. Returns (outputs[B,T,128,256], h_last, c_last).

Per-core mapping (2 samples/core):
  * conv expressed as PSUM-accumulated matmuls: contraction 960 = 192ch x 5taps,
    done as 8 matmuls of contract-128 per 128-wide output chunk (4 chunks),
    free dim N = 2 samples x 256 = 512 (one full PSUM bank).
  * h-part (128 ch): 5 tap-shifted matmuls against a zero-padded persistent
    h buffer [128, 2, 260]; tap shift = free-dim AP offset (free).
  * x-part (64 ch): two taps packed per matmul: x stored twice in one tile
    (partitions 0-63 unshifted, 64-127 shifted by one column) -> 3 matmuls,
    with tap-paired weight layout prepared host-side.
  * fp32r matmuls (full PE rate at free dim >= 256), fp32 PSUM accumulate.
  * gates: ScalarE activation with fused per-partition bias (sigmoid/tanh are
    in one ACT table set); cell update on VectorE.
"""

import sys

import numpy as np

for _p in ("/opt/trn_rl_repo", "/root/.axon_site/_ro/trn_rl_repo"):
    if _p not in sys.path:
        sys.path.append(_p)

HIDDEN = 128
B, T, C, W = 16, 64, 64, 256
KTAPS = 5
PAD = KTAPS // 2           # 2
WP = W + 2 * PAD           # 260
NCORES = 8
BPC = B // NCORES          # 2 samples per core
NM = 4                     # output-channel chunks of 128 (i, f, o, g)

TRACE = False              # test.py sets True to capture NTFF profile
TRACE_KWARGS = {}
LAST_RESULT = None         # BassKernelResults of the last run (for timing)

_BUILT = {}


def _build(t_steps=T):
    """Build + compile the single-core Bass program (SPMD across cores)."""
    import concourse.bacc as bacc
    import concourse.mybir as mybir
    import concourse.tile as tile

    f32 = mybir.dt.float32
    f32r = mybir.dt.float32r
    AF = mybir.ActivationFunctionType

    nc = bacc.Bacc("TRN2", target_bir_lowering=False, debug=False)

    # matmul operands are float32r (fp32 RNE-rounded to 11 mantissa bits);
    # x/weights are pre-rounded host-side so their DMAs are plain byte copies.
    xh_d = nc.dram_tensor("xh", [t_steps, C, BPC, W], f32r, kind="ExternalInput")
    wh_d = nc.dram_tensor("wh", [HIDDEN, NM * KTAPS * 128], f32r, kind="ExternalInput")
    wx_d = nc.dram_tensor("wx", [128, NM * 3 * 128], f32r, kind="ExternalInput")
    b_d = nc.dram_tensor("bias", [128, NM], f32, kind="ExternalInput")
    out_d = nc.dram_tensor("out", [t_steps, HIDDEN, BPC, W], f32r, kind="ExternalOutput")
    cout_d = nc.dram_tensor("c_out", [HIDDEN, BPC, W], f32, kind="ExternalOutput")

    xh, wh, wx, bias, out, cout = (
        h.ap() for h in (xh_d, wh_d, wx_d, b_d, out_d, cout_d)
    )

    # matmul emission order of gate chunks: o (chunk 2) last so the
    # sig(o)*tanh(c) tail starts as early as possible.
    MORDER = [0, 1, 3, 2]

    with tile.TileContext(nc) as tc:
        with tc.tile_pool(name="wpool", bufs=1) as wpool, \
             tc.tile_pool(name="spool", bufs=1) as spool, \
             tc.tile_pool(name="gpool", bufs=3) as gpool, \
             tc.tile_pool(name="pspool", bufs=8, space="PSUM") as pspool:

            # --- weights + bias (resident for the whole kernel) ---
            # one big tile per weight group -> one large contiguous DMA each
            wh_sb = wpool.tile([HIDDEN, NM * KTAPS * 128], f32r, name="wh_sb")
            wx_sb = wpool.tile([128, NM * 3 * 128], f32r, name="wx_sb")
            b_sb = wpool.tile([128, NM], f32, name="b_sb")
            nc.sync.dma_start(out=wx_sb, in_=wx)
            nc.sync.dma_start(out=b_sb, in_=bias)
            nc.sync.dma_start(out=wh_sb, in_=wh)
            wh_t = [
                [wh_sb[:, (m * KTAPS + k) * 128:(m * KTAPS + k + 1) * 128]
                 for k in range(KTAPS)]
                for m in range(NM)
            ]
            wx_t = [
                [wx_sb[:, (m * 3 + p) * 128:(m * 3 + p + 1) * 128]
                 for p in range(3)]
                for m in range(NM)
            ]
            b_t = [b_sb[:, m:m + 1] for m in range(NM)]

            # --- persistent state ---
            NXBUF = 3
            x2 = [spool.tile([128, BPC, WP], f32r, name=f"x2_{i}") for i in range(NXBUF)]
            hp = [spool.tile([HIDDEN, BPC, WP], f32r, name=f"hp_{i}") for i in range(2)]
            c_st = spool.tile([HIDDEN, BPC, W], f32, name="c_st")
            for buf in (*x2, *hp):
                nc.gpsimd.memset(buf.bitcast(mybir.dt.uint32), 0)
            nc.gpsimd.memset(c_st, 0.0)

            for t in range(t_steps):
                xt = x2[t % NXBUF]
                # top half: x padded by 2; bottom half: same shifted left by 1
                nc.sync.dma_start(out=xt[0:C, :, PAD:PAD + W], in_=xh[t])
                nc.sync.dma_start(out=xt[C:2 * C, :, PAD - 1:PAD - 1 + W], in_=xh[t])
                h_in = hp[t % 2]
                h_out = hp[(t + 1) % 2]

                ps = {
                    m: pspool.tile([128, BPC, W], f32, tag="ps", name=f"ps_{t}_{m}")
                    for m in MORDER
                }
                # x-part first: independent of h, keeps PE busy while the
                # previous step's elementwise tail computes h.
                for m in MORDER:
                    for p in range(3):
                        nc.tensor.matmul(
                            ps[m],
                            wx_t[m][p],
                            xt[:, :, 2 * p:2 * p + W],
                            start=(p == 0),
                            stop=False,
                        )
                for m in MORDER:
                    for k in range(KTAPS):
                        nc.tensor.matmul(
                            ps[m],
                            wh_t[m][k],
                            h_in[:, :, k:k + W],
                            start=False,
                            stop=(k == KTAPS - 1),
                        )

                sig_i = gpool.tile([128, BPC, W], f32, tag="si", name=f"si_{t}")
                sig_f = gpool.tile([128, BPC, W], f32, tag="sf", name=f"sf_{t}")
                tanh_g = gpool.tile([128, BPC, W], f32, tag="tg", name=f"tg_{t}")
                sig_o = gpool.tile([128, BPC, W], f32, tag="so", name=f"so_{t}")
                nc.scalar.activation(out=sig_i, in_=ps[0], func=AF.Sigmoid, bias=b_t[0])
                nc.scalar.activation(out=sig_f, in_=ps[1], func=AF.Sigmoid, bias=b_t[1])
                nc.scalar.activation(out=tanh_g, in_=ps[3], func=AF.Tanh, bias=b_t[3])
                nc.scalar.activation(out=sig_o, in_=ps[2], func=AF.Sigmoid, bias=b_t[2])

                ig = gpool.tile([128, BPC, W], f32, tag="ig", name=f"ig_{t}")
                fc = gpool.tile([128, BPC, W], f32, tag="fc", name=f"fc_{t}")
                nc.vector.tensor_mul(ig, sig_i, tanh_g)
                nc.vector.tensor_mul(fc, sig_f, c_st)
                nc.vector.tensor_add(c_st, ig, fc)
                tnc = gpool.tile([128, BPC, W], f32, tag="tnc", name=f"tnc_{t}")
                nc.scalar.activation(out=tnc, in_=c_st, func=AF.Tanh)
                nc.vector.tensor_mul(h_out[:, :, PAD:PAD + W], sig_o, tnc)
                nc.sync.dma_start(out=out[t], in_=h_out[:, :, PAD:PAD + W])

            nc.sync.dma_start(out=cout, in_=c_st)

    nc.compile()
    return nc


def get_built(t_steps=T):
    if t_steps not in _BUILT:
        _BUILT[t_steps] = _build(t_steps)
    return _BUILT[t_steps]


def round_fp32r(a):
    """RNE-round fp32 to the fp32r grid (11 mantissa bits, low 12 bits zero)."""
    u = np.ascontiguousarray(a, np.float32).view(np.uint32)
    u = (u + np.uint32(0x7FF) + ((u >> np.uint32(12)) & np.uint32(1))) \
        & np.uint32(0xFFFFF000)
    return u.view(np.float32)


def prep_weights(W_conv, b_conv):
    """Host-side weight layout: transposed lhsT tiles + tap-paired x weights,
    packed into single [128, ncols] SBUF images (one DMA each)."""
    Wc = round_fp32r(np.asarray(W_conv, dtype=np.float32))
    bc = np.asarray(b_conv, dtype=np.float32)
    W4 = Wc.reshape(NM, 128, C + HIDDEN, KTAPS)            # [m, co, ci, k]
    wh4 = W4[:, :, C:, :].transpose(0, 3, 2, 1)            # [m,k,ci(128),co(128)]
    wh = np.ascontiguousarray(
        wh4.reshape(NM * KTAPS, HIDDEN, 128).transpose(1, 0, 2).reshape(
            HIDDEN, NM * KTAPS * 128)
    )
    Wx4 = W4[:, :, :C, :]                                   # [m, co, 64, 5]
    wx4 = np.zeros((NM, 3, 128, 128), np.float32)
    for p in range(2):
        wx4[:, p, 0:C, :] = Wx4[:, :, :, 2 * p].transpose(0, 2, 1)
        wx4[:, p, C:2 * C, :] = Wx4[:, :, :, 2 * p + 1].transpose(0, 2, 1)
    wx4[:, 2, 0:C, :] = Wx4[:, :, :, 4].transpose(0, 2, 1)
    wx = np.ascontiguousarray(
        wx4.reshape(NM * 3, 128, 128).transpose(1, 0, 2).reshape(128, NM * 3 * 128)
    )
    bias = np.ascontiguousarray(bc.reshape(NM, 128).T)      # [128, NM]
    return wh, wx, bias


def make_in_maps(x, W_conv, b_conv, t_steps=T):
    x = round_fp32r(np.asarray(x, dtype=np.float32))
    wh, wx, bias = prep_weights(W_conv, b_conv)
    in_maps = []
    for cid in range(NCORES):
        xs = x[cid * BPC:(cid + 1) * BPC, :t_steps]         # [2, T', C, W]
        xh = np.ascontiguousarray(xs.transpose(1, 2, 0, 3))  # [T', C, 2, W]
        in_maps.append({"xh": xh, "wh": wh, "wx": wx, "bias": bias})
    return in_maps


def assemble(results, t_steps=T):
    outputs = np.empty((B, t_steps, HIDDEN, W), np.float32)
    c_full = np.empty((B, HIDDEN, W), np.float32)
    for cid in range(NCORES):
        o = results[cid]["out"]                              # [T', 128, 2, W]
        outputs[cid * BPC:(cid + 1) * BPC] = o.transpose(2, 0, 1, 3)
        c_full[cid * BPC:(cid + 1) * BPC] = results[cid]["c_out"].transpose(1, 0, 2)
    h_full = outputs[:, -1].copy()
    return outputs, h_full, c_full


def _install_ntff_shim():
    """The image's antenv package lacks axon_hooks; recreate it so
    run_bass_kernel_spmd(trace=True) can capture NTFF profiles."""
    import types

    if "antenv.axon_hooks" in sys.modules:
        return
    try:
        import antenv
        from trn_agent_boot.trn_boot import _ntff_profile_via_ctypes
    except ImportError:
        return
    hooks = types.ModuleType("antenv.axon_hooks")
    hooks._hook = _ntff_profile_via_ctypes("/opt/axon/libaxon_pjrt.so")
    hooks.get_axon_ntff_profile_hook = lambda: hooks._hook
    hooks.set_axon_ntff_profile_hook = lambda h: setattr(hooks, "_hook", h)
    sys.modules["antenv.axon_hooks"] = hooks
    antenv.axon_hooks = hooks


def kernel(x, W_conv, b_conv):
    global LAST_RESULT
    from concourse import bass_utils

    if TRACE:
        _install_ntff_shim()

    nc = get_built(T)
    in_maps = make_in_maps(x, W_conv, b_conv, T)
    res = bass_utils.run_bass_kernel_spmd(
        nc, in_maps, core_ids=list(range(NCORES)), trace=TRACE, **TRACE_KWARGS
    )
    LAST_RESULT = res
    return assemble(res.results, T)


# revision 19
# speedup vs baseline: 1.0884x; 1.0563x over previous
"""ConvLSTM Trainium2 kernel (Bass/Tile), data-parallel over batch on 8 cores.

Problem shapes (hardcoded): x[16,64,64,256] f32, W_conv[512,192,5], b_conv[512].
reference: per step t, conv1d(concat(x_t, h), W_conv) 'same' pad -> 4 gates ->
LSTM cell update. Returns (outputs[B,T,128,256], h_last, c_last).

Per-core mapping (2 samples/core):
  * conv expressed as PSUM-accumulated matmuls: contraction 960 = 192ch x 5taps,
    done as 8 matmuls of contract-128 per 128-wide output chunk (4 chunks),
    free dim N = 2 samples x 256 = 512 (one full PSUM bank).
  * h-part (128 ch): 5 tap-shifted matmuls against a zero-padded persistent
    h buffer [128, 2, 260]; tap shift = free-dim AP offset (free).
  * x-part (64 ch): two taps packed per matmul: x stored twice in one tile
    (partitions 0-63 unshifted, 64-127 shifted by one column) -> 3 matmuls,
    with tap-paired weight layout prepared host-side.
  * matmul operands in MMDT (bf16 default, fp32r option), fp32 PSUM accumulate.
  * gates: ScalarE activation with fused per-partition bias (sigmoid/tanh are
    in one ACT table set); cell update on VectorE.
"""

import sys

import numpy as np

for _p in ("/opt/trn_rl_repo", "/root/.axon_site/_ro/trn_rl_repo"):
    if _p not in sys.path:
        sys.path.append(_p)

HIDDEN = 128
B, T, C, W = 16, 64, 64, 256
KTAPS = 5
PAD = KTAPS // 2           # 2
WP = W + 2 * PAD           # 260
NCORES = 8
BPC = B // NCORES          # 2 samples per core
NM = 4                     # output-channel chunks of 128 (i, f, o, g)

TRACE = False              # test.py sets True to capture NTFF profile
TRACE_KWARGS = {}
LAST_RESULT = None         # BassKernelResults of the last run (for timing)
MMDT = "bf16"              # matmul operand dtype: "bf16" (FWL) or "f32r"

_BUILT = {}


def _build(t_steps=T, mmdt=None):
    """Build + compile the single-core Bass program (SPMD across cores)."""
    import concourse.bacc as bacc
    import concourse.mybir as mybir
    import concourse.tile as tile

    f32 = mybir.dt.float32
    AF = mybir.ActivationFunctionType
    if mmdt is None:
        mmdt = MMDT
    # matmul operand dtype: float32r = fp32 RNE-rounded to 11 mantissa bits
    # (full PE rate at free dim >= 256, but no fast-weight-load); bfloat16
    # gets FWL so back-to-back matmul spacing is tighter.
    mdt = mybir.dt.bfloat16 if mmdt == "bf16" else mybir.dt.float32r

    nc = bacc.Bacc("TRN2", target_bir_lowering=False, debug=False)

    # x/weights are pre-rounded host-side so their DMAs are plain byte copies.
    xh_d = nc.dram_tensor("xh", [t_steps, C, BPC, W], mdt, kind="ExternalInput")
    wh_d = nc.dram_tensor("wh", [HIDDEN, NM * KTAPS * 128], mdt, kind="ExternalInput")
    wx_d = nc.dram_tensor("wx", [128, NM * 3 * 128], mdt, kind="ExternalInput")
    b_d = nc.dram_tensor("bias", [128, NM], f32, kind="ExternalInput")
    out_d = nc.dram_tensor("out", [t_steps, HIDDEN, BPC, W], mdt, kind="ExternalOutput")
    cout_d = nc.dram_tensor("c_out", [HIDDEN, BPC, W], f32, kind="ExternalOutput")

    xh, wh, wx, bias, out, cout = (
        h.ap() for h in (xh_d, wh_d, wx_d, b_d, out_d, cout_d)
    )

    # matmul emission order of gate chunks: o (chunk 2) last so the
    # sig(o)*tanh(c) tail starts as early as possible.
    MORDER = [0, 1, 3, 2]

    with tile.TileContext(nc) as tc:
        with tc.tile_pool(name="wpool", bufs=1) as wpool, \
             tc.tile_pool(name="spool", bufs=1) as spool, \
             tc.tile_pool(name="gpool", bufs=3) as gpool, \
             tc.tile_pool(name="pspool", bufs=8, space="PSUM") as pspool:

            # --- weights + bias (resident for the whole kernel) ---
            # one big tile per weight group -> one large contiguous DMA each
            wh_sb = wpool.tile([HIDDEN, NM * KTAPS * 128], mdt, name="wh_sb")
            wx_sb = wpool.tile([128, NM * 3 * 128], mdt, name="wx_sb")
            b_sb = wpool.tile([128, NM], f32, name="b_sb")
            nc.sync.dma_start(out=wx_sb, in_=wx)
            nc.sync.dma_start(out=b_sb, in_=bias)
            nc.sync.dma_start(out=wh_sb, in_=wh)
            wh_t = [
                [wh_sb[:, (m * KTAPS + k) * 128:(m * KTAPS + k + 1) * 128]
                 for k in range(KTAPS)]
                for m in range(NM)
            ]
            wx_t = [
                [wx_sb[:, (m * 3 + p) * 128:(m * 3 + p + 1) * 128]
                 for p in range(3)]
                for m in range(NM)
            ]
            b_t = [b_sb[:, m:m + 1] for m in range(NM)]

            # --- persistent state ---
            NXBUF = 3
            x2 = [spool.tile([128, BPC, WP], mdt, name=f"x2_{i}") for i in range(NXBUF)]
            hp = [spool.tile([HIDDEN, BPC, WP], mdt, name=f"hp_{i}") for i in range(2)]
            c_st = spool.tile([HIDDEN, BPC, W], f32, name="c_st")
            for buf in (*x2, *hp):
                nc.gpsimd.memset(buf.bitcast(mybir.dt.uint32), 0)
            nc.gpsimd.memset(c_st, 0.0)

            for t in range(t_steps):
                xt = x2[t % NXBUF]
                # top half: x padded by 2; bottom half: same shifted left by 1
                nc.sync.dma_start(out=xt[0:C, :, PAD:PAD + W], in_=xh[t])
                nc.sync.dma_start(out=xt[C:2 * C, :, PAD - 1:PAD - 1 + W], in_=xh[t])
                h_in = hp[t % 2]
                h_out = hp[(t + 1) % 2]

                ps = {
                    m: pspool.tile([128, BPC, W], f32, tag="ps", name=f"ps_{t}_{m}")
                    for m in MORDER
                }
                # x-part first: independent of h, keeps PE busy while the
                # previous step's elementwise tail computes h.
                for m in MORDER:
                    for p in range(3):
                        nc.tensor.matmul(
                            ps[m],
                            wx_t[m][p],
                            xt[:, :, 2 * p:2 * p + W],
                            start=(p == 0),
                            stop=False,
                        )
                for m in MORDER:
                    for k in range(KTAPS):
                        nc.tensor.matmul(
                            ps[m],
                            wh_t[m][k],
                            h_in[:, :, k:k + W],
                            start=False,
                            stop=(k == KTAPS - 1),
                        )

                sig_i = gpool.tile([128, BPC, W], f32, tag="si", name=f"si_{t}")
                sig_f = gpool.tile([128, BPC, W], f32, tag="sf", name=f"sf_{t}")
                tanh_g = gpool.tile([128, BPC, W], f32, tag="tg", name=f"tg_{t}")
                sig_o = gpool.tile([128, BPC, W], f32, tag="so", name=f"so_{t}")
                nc.scalar.activation(out=sig_i, in_=ps[0], func=AF.Sigmoid, bias=b_t[0])
                nc.scalar.activation(out=sig_f, in_=ps[1], func=AF.Sigmoid, bias=b_t[1])
                nc.scalar.activation(out=tanh_g, in_=ps[3], func=AF.Tanh, bias=b_t[3])
                nc.scalar.activation(out=sig_o, in_=ps[2], func=AF.Sigmoid, bias=b_t[2])

                ig = gpool.tile([128, BPC, W], f32, tag="ig", name=f"ig_{t}")
                fc = gpool.tile([128, BPC, W], f32, tag="fc", name=f"fc_{t}")
                nc.vector.tensor_mul(ig, sig_i, tanh_g)
                nc.vector.tensor_mul(fc, sig_f, c_st)
                nc.vector.tensor_add(c_st, ig, fc)
                tnc = gpool.tile([128, BPC, W], f32, tag="tnc", name=f"tnc_{t}")
                nc.scalar.activation(out=tnc, in_=c_st, func=AF.Tanh)
                nc.vector.tensor_mul(h_out[:, :, PAD:PAD + W], sig_o, tnc)
                nc.sync.dma_start(out=out[t], in_=h_out[:, :, PAD:PAD + W])

            nc.sync.dma_start(out=cout, in_=c_st)

    nc.compile()
    return nc


def get_built(t_steps=T, mmdt=None):
    mmdt = MMDT if mmdt is None else mmdt
    key = (t_steps, mmdt)
    if key not in _BUILT:
        _BUILT[key] = _build(t_steps, mmdt)
    return _BUILT[key]


def round_fp32r(a):
    """RNE-round fp32 to the fp32r grid (11 mantissa bits, low 12 bits zero)."""
    u = np.ascontiguousarray(a, np.float32).view(np.uint32)
    u = (u + np.uint32(0x7FF) + ((u >> np.uint32(12)) & np.uint32(1))) \
        & np.uint32(0xFFFFF000)
    return u.view(np.float32)


def _mm_cast(a, mmdt):
    if mmdt == "bf16":
        import ml_dtypes

        return np.asarray(a, np.float32).astype(ml_dtypes.bfloat16)
    return round_fp32r(np.asarray(a, dtype=np.float32))


def prep_weights(W_conv, b_conv, mmdt=None):
    """Host-side weight layout: transposed lhsT tiles + tap-paired x weights,
    packed into single [128, ncols] SBUF images (one DMA each)."""
    mmdt = MMDT if mmdt is None else mmdt
    Wc = np.asarray(W_conv, dtype=np.float32)
    bc = np.asarray(b_conv, dtype=np.float32)
    W4 = Wc.reshape(NM, 128, C + HIDDEN, KTAPS)            # [m, co, ci, k]
    wh4 = W4[:, :, C:, :].transpose(0, 3, 2, 1)            # [m,k,ci(128),co(128)]
    wh = np.ascontiguousarray(
        wh4.reshape(NM * KTAPS, HIDDEN, 128).transpose(1, 0, 2).reshape(
            HIDDEN, NM * KTAPS * 128)
    )
    Wx4 = W4[:, :, :C, :]                                   # [m, co, 64, 5]
    wx4 = np.zeros((NM, 3, 128, 128), np.float32)
    for p in range(2):
        wx4[:, p, 0:C, :] = Wx4[:, :, :, 2 * p].transpose(0, 2, 1)
        wx4[:, p, C:2 * C, :] = Wx4[:, :, :, 2 * p + 1].transpose(0, 2, 1)
    wx4[:, 2, 0:C, :] = Wx4[:, :, :, 4].transpose(0, 2, 1)
    wx = np.ascontiguousarray(
        wx4.reshape(NM * 3, 128, 128).transpose(1, 0, 2).reshape(128, NM * 3 * 128)
    )
    bias = np.ascontiguousarray(bc.reshape(NM, 128).T)      # [128, NM]
    return _mm_cast(wh, mmdt), _mm_cast(wx, mmdt), bias


def make_in_maps(x, W_conv, b_conv, t_steps=T, mmdt=None):
    mmdt = MMDT if mmdt is None else mmdt
    x = _mm_cast(np.asarray(x, dtype=np.float32), mmdt)
    wh, wx, bias = prep_weights(W_conv, b_conv, mmdt)
    in_maps = []
    for cid in range(NCORES):
        xs = x[cid * BPC:(cid + 1) * BPC, :t_steps]         # [2, T', C, W]
        xh = np.ascontiguousarray(xs.transpose(1, 2, 0, 3))  # [T', C, 2, W]
        in_maps.append({"xh": xh, "wh": wh, "wx": wx, "bias": bias})
    return in_maps


def assemble(results, t_steps=T):
    outputs = np.empty((B, t_steps, HIDDEN, W), np.float32)
    c_full = np.empty((B, HIDDEN, W), np.float32)
    for cid in range(NCORES):
        o = np.asarray(results[cid]["out"], np.float32)      # [T', 128, 2, W]
        outputs[cid * BPC:(cid + 1) * BPC] = o.transpose(2, 0, 1, 3)
        c_full[cid * BPC:(cid + 1) * BPC] = results[cid]["c_out"].transpose(1, 0, 2)
    h_full = outputs[:, -1].copy()
    return outputs, h_full, c_full


def _install_ntff_shim():
    """The image's antenv package lacks axon_hooks; recreate it so
    run_bass_kernel_spmd(trace=True) can capture NTFF profiles."""
    import types

    if "antenv.axon_hooks" in sys.modules:
        return
    try:
        import antenv
        from trn_agent_boot.trn_boot import _ntff_profile_via_ctypes
    except ImportError:
        return
    hooks = types.ModuleType("antenv.axon_hooks")
    hooks._hook = _ntff_profile_via_ctypes("/opt/axon/libaxon_pjrt.so")
    hooks.get_axon_ntff_profile_hook = lambda: hooks._hook
    hooks.set_axon_ntff_profile_hook = lambda h: setattr(hooks, "_hook", h)
    sys.modules["antenv.axon_hooks"] = hooks
    antenv.axon_hooks = hooks


def kernel(x, W_conv, b_conv):
    global LAST_RESULT
    from concourse import bass_utils

    if TRACE:
        _install_ntff_shim()

    nc = get_built(T)
    in_maps = make_in_maps(x, W_conv, b_conv, T)
    res = bass_utils.run_bass_kernel_spmd(
        nc, in_maps, core_ids=list(range(NCORES)), trace=TRACE, **TRACE_KWARGS
    )
    LAST_RESULT = res
    return assemble(res.results, T)


# revision 20
# speedup vs baseline: 1.0961x; 1.0070x over previous
"""ConvLSTM Trainium2 kernel (Bass/Tile), data-parallel over batch on 8 cores.

Problem shapes (hardcoded): x[16,64,64,256] f32, W_conv[512,192,5], b_conv[512].
reference: per step t, conv1d(concat(x_t, h), W_conv) 'same' pad -> 4 gates ->
LSTM cell update. Returns (outputs[B,T,128,256], h_last, c_last).

Per-core mapping (2 samples/core):
  * conv expressed as PSUM-accumulated matmuls: contraction 960 = 192ch x 5taps,
    done as 8 matmuls of contract-128 per 128-wide output chunk (4 chunks),
    free dim N = 2 samples x 256 = 512 (one full PSUM bank).
  * h-part (128 ch): 5 tap-shifted matmuls against a zero-padded persistent
    h buffer [128, 2, 260]; tap shift = free-dim AP offset (free).
  * x-part (64 ch): two taps packed per matmul: x stored twice in one tile
    (partitions 0-63 unshifted, 64-127 shifted by one column) -> 3 matmuls,
    with tap-paired weight layout prepared host-side.
  * matmul operands in MMDT (bf16 default, fp32r option), fp32 PSUM accumulate.
  * gates: ScalarE activation with fused per-partition bias (sigmoid/tanh are
    in one ACT table set); cell update on VectorE.
"""

import sys

import numpy as np

for _p in ("/opt/trn_rl_repo", "/root/.axon_site/_ro/trn_rl_repo"):
    if _p not in sys.path:
        sys.path.append(_p)

HIDDEN = 128
B, T, C, W = 16, 64, 64, 256
KTAPS = 5
PAD = KTAPS // 2           # 2
WP = W + 2 * PAD           # 260
NCORES = 8
BPC = B // NCORES          # 2 samples per core
NM = 4                     # output-channel chunks of 128 (i, f, o, g)

TRACE = False              # test.py sets True to capture NTFF profile
TRACE_KWARGS = {}
LAST_RESULT = None         # BassKernelResults of the last run (for timing)
MMDT = "bf16"              # matmul operand dtype: "bf16" (FWL) or "f32r"

_BUILT = {}


def _build(t_steps=T, mmdt=None):
    """Build + compile the single-core Bass program (SPMD across cores)."""
    import concourse.bacc as bacc
    import concourse.mybir as mybir
    import concourse.tile as tile

    f32 = mybir.dt.float32
    AF = mybir.ActivationFunctionType
    if mmdt is None:
        mmdt = MMDT
    # matmul operand dtype: float32r = fp32 RNE-rounded to 11 mantissa bits
    # (full PE rate at free dim >= 256, but no fast-weight-load); bfloat16
    # gets FWL so back-to-back matmul spacing is tighter.
    mdt = mybir.dt.bfloat16 if mmdt == "bf16" else mybir.dt.float32r

    nc = bacc.Bacc("TRN2", target_bir_lowering=False, debug=False)

    # x/weights are pre-rounded host-side so their DMAs are plain byte copies.
    xh_d = nc.dram_tensor("xh", [t_steps, C, BPC, W], mdt, kind="ExternalInput")
    wh_d = nc.dram_tensor("wh", [HIDDEN, NM * KTAPS * 128], mdt, kind="ExternalInput")
    wx_d = nc.dram_tensor("wx", [128, NM * 3 * 128], mdt, kind="ExternalInput")
    b_d = nc.dram_tensor("bias", [128, NM], f32, kind="ExternalInput")
    out_d = nc.dram_tensor("out", [t_steps, HIDDEN, BPC, W], mdt, kind="ExternalOutput")
    cout_d = nc.dram_tensor("c_out", [HIDDEN, BPC, W], f32, kind="ExternalOutput")

    xh, wh, wx, bias, out, cout = (
        h.ap() for h in (xh_d, wh_d, wx_d, b_d, out_d, cout_d)
    )

    # matmul emission order of gate chunks: o (chunk 2) last so the
    # sig(o)*tanh(c) tail starts as early as possible.
    MORDER = [0, 1, 3, 2]

    with tile.TileContext(nc) as tc:
        with tc.tile_pool(name="wpool", bufs=1) as wpool, \
             tc.tile_pool(name="spool", bufs=1) as spool, \
             tc.tile_pool(name="gpool", bufs=3) as gpool, \
             tc.tile_pool(name="pspool", bufs=8, space="PSUM") as pspool:

            # --- weights + bias (resident for the whole kernel) ---
            # one big tile per weight group -> one large contiguous DMA each
            wh_sb = wpool.tile([HIDDEN, NM * KTAPS * 128], mdt, name="wh_sb")
            wx_sb = wpool.tile([128, NM * 3 * 128], mdt, name="wx_sb")
            b_sb = wpool.tile([128, NM], f32, name="b_sb")
            nc.sync.dma_start(out=wx_sb, in_=wx)
            nc.sync.dma_start(out=b_sb, in_=bias)
            nc.sync.dma_start(out=wh_sb, in_=wh)
            wh_t = [
                [wh_sb[:, (m * KTAPS + k) * 128:(m * KTAPS + k + 1) * 128]
                 for k in range(KTAPS)]
                for m in range(NM)
            ]
            wx_t = [
                [wx_sb[:, (m * 3 + p) * 128:(m * 3 + p + 1) * 128]
                 for p in range(3)]
                for m in range(NM)
            ]
            b_t = [b_sb[:, m:m + 1] for m in range(NM)]

            # --- persistent state ---
            NXBUF = 3
            x2 = [spool.tile([128, BPC, WP], mdt, name=f"x2_{i}") for i in range(NXBUF)]
            hp = [spool.tile([HIDDEN, BPC, WP], mdt, name=f"hp_{i}") for i in range(2)]
            c_st = spool.tile([HIDDEN, BPC, W], f32, name="c_st")
            for buf in (*x2, *hp):
                nc.gpsimd.memset(buf.bitcast(mybir.dt.uint32), 0)
            nc.gpsimd.memset(c_st, 0.0)

            for t in range(t_steps):
                xt = x2[t % NXBUF]
                # top half: x padded by 2; bottom half: same shifted left by 1
                nc.sync.dma_start(out=xt[0:C, :, PAD:PAD + W], in_=xh[t])
                nc.sync.dma_start(out=xt[C:2 * C, :, PAD - 1:PAD - 1 + W], in_=xh[t])
                h_in = hp[t % 2]
                h_out = hp[(t + 1) % 2]

                ps = {
                    m: pspool.tile([128, BPC, W], f32, tag="ps", name=f"ps_{t}_{m}")
                    for m in MORDER
                }
                # x-part first: independent of h, keeps PE busy while the
                # previous step's elementwise tail computes h.
                # t == 0: h is all-zero, skip its matmuls entirely.
                for m in MORDER:
                    for p in range(3):
                        nc.tensor.matmul(
                            ps[m],
                            wx_t[m][p],
                            xt[:, :, 2 * p:2 * p + W],
                            start=(p == 0),
                            stop=(t == 0 and p == 2),
                        )
                if t > 0:
                    for m in MORDER:
                        for k in range(KTAPS):
                            nc.tensor.matmul(
                                ps[m],
                                wh_t[m][k],
                                h_in[:, :, k:k + W],
                                start=False,
                                stop=(k == KTAPS - 1),
                            )

                sig_i = gpool.tile([128, BPC, W], f32, tag="si", name=f"si_{t}")
                sig_f = gpool.tile([128, BPC, W], f32, tag="sf", name=f"sf_{t}")
                tanh_g = gpool.tile([128, BPC, W], f32, tag="tg", name=f"tg_{t}")
                sig_o = gpool.tile([128, BPC, W], f32, tag="so", name=f"so_{t}")
                nc.scalar.activation(out=sig_i, in_=ps[0], func=AF.Sigmoid, bias=b_t[0])
                nc.scalar.activation(out=sig_f, in_=ps[1], func=AF.Sigmoid, bias=b_t[1])
                nc.scalar.activation(out=tanh_g, in_=ps[3], func=AF.Tanh, bias=b_t[3])
                nc.scalar.activation(out=sig_o, in_=ps[2], func=AF.Sigmoid, bias=b_t[2])

                ig = gpool.tile([128, BPC, W], f32, tag="ig", name=f"ig_{t}")
                fc = gpool.tile([128, BPC, W], f32, tag="fc", name=f"fc_{t}")
                nc.vector.tensor_mul(ig, sig_i, tanh_g)
                nc.vector.tensor_mul(fc, sig_f, c_st)
                nc.vector.tensor_add(c_st, ig, fc)
                tnc = gpool.tile([128, BPC, W], f32, tag="tnc", name=f"tnc_{t}")
                nc.scalar.activation(out=tnc, in_=c_st, func=AF.Tanh)
                nc.vector.tensor_mul(h_out[:, :, PAD:PAD + W], sig_o, tnc)
                nc.sync.dma_start(out=out[t], in_=h_out[:, :, PAD:PAD + W])

            nc.sync.dma_start(out=cout, in_=c_st)

    nc.compile()
    return nc


def get_built(t_steps=T, mmdt=None):
    mmdt = MMDT if mmdt is None else mmdt
    key = (t_steps, mmdt)
    if key not in _BUILT:
        _BUILT[key] = _build(t_steps, mmdt)
    return _BUILT[key]


def round_fp32r(a):
    """RNE-round fp32 to the fp32r grid (11 mantissa bits, low 12 bits zero)."""
    u = np.ascontiguousarray(a, np.float32).view(np.uint32)
    u = (u + np.uint32(0x7FF) + ((u >> np.uint32(12)) & np.uint32(1))) \
        & np.uint32(0xFFFFF000)
    return u.view(np.float32)


def _mm_cast(a, mmdt):
    if mmdt == "bf16":
        import ml_dtypes

        return np.asarray(a, np.float32).astype(ml_dtypes.bfloat16)
    return round_fp32r(np.asarray(a, dtype=np.float32))


def prep_weights(W_conv, b_conv, mmdt=None):
    """Host-side weight layout: transposed lhsT tiles + tap-paired x weights,
    packed into single [128, ncols] SBUF images (one DMA each)."""
    mmdt = MMDT if mmdt is None else mmdt
    Wc = np.asarray(W_conv, dtype=np.float32)
    bc = np.asarray(b_conv, dtype=np.float32)
    W4 = Wc.reshape(NM, 128, C + HIDDEN, KTAPS)            # [m, co, ci, k]
    wh4 = W4[:, :, C:, :].transpose(0, 3, 2, 1)            # [m,k,ci(128),co(128)]
    wh = np.ascontiguousarray(
        wh4.reshape(NM * KTAPS, HIDDEN, 128).transpose(1, 0, 2).reshape(
            HIDDEN, NM * KTAPS * 128)
    )
    Wx4 = W4[:, :, :C, :]                                   # [m, co, 64, 5]
    wx4 = np.zeros((NM, 3, 128, 128), np.float32)
    for p in range(2):
        wx4[:, p, 0:C, :] = Wx4[:, :, :, 2 * p].transpose(0, 2, 1)
        wx4[:, p, C:2 * C, :] = Wx4[:, :, :, 2 * p + 1].transpose(0, 2, 1)
    wx4[:, 2, 0:C, :] = Wx4[:, :, :, 4].transpose(0, 2, 1)
    wx = np.ascontiguousarray(
        wx4.reshape(NM * 3, 128, 128).transpose(1, 0, 2).reshape(128, NM * 3 * 128)
    )
    bias = np.ascontiguousarray(bc.reshape(NM, 128).T)      # [128, NM]
    return _mm_cast(wh, mmdt), _mm_cast(wx, mmdt), bias


def make_in_maps(x, W_conv, b_conv, t_steps=T, mmdt=None):
    mmdt = MMDT if mmdt is None else mmdt
    x = _mm_cast(np.asarray(x, dtype=np.float32), mmdt)
    wh, wx, bias = prep_weights(W_conv, b_conv, mmdt)
    in_maps = []
    for cid in range(NCORES):
        xs = x[cid * BPC:(cid + 1) * BPC, :t_steps]         # [2, T', C, W]
        xh = np.ascontiguousarray(xs.transpose(1, 2, 0, 3))  # [T', C, 2, W]
        in_maps.append({"xh": xh, "wh": wh, "wx": wx, "bias": bias})
    return in_maps


def assemble(results, t_steps=T):
    outputs = np.empty((B, t_steps, HIDDEN, W), np.float32)
    c_full = np.empty((B, HIDDEN, W), np.float32)
    for cid in range(NCORES):
        o = np.asarray(results[cid]["out"], np.float32)      # [T', 128, 2, W]
        outputs[cid * BPC:(cid + 1) * BPC] = o.transpose(2, 0, 1, 3)
        c_full[cid * BPC:(cid + 1) * BPC] = results[cid]["c_out"].transpose(1, 0, 2)
    h_full = outputs[:, -1].copy()
    return outputs, h_full, c_full


def _install_ntff_shim():
    """The image's antenv package lacks axon_hooks; recreate it so
    run_bass_kernel_spmd(trace=True) can capture NTFF profiles."""
    import types

    if "antenv.axon_hooks" in sys.modules:
        return
    try:
        import antenv
        from trn_agent_boot.trn_boot import _ntff_profile_via_ctypes
    except ImportError:
        return
    hooks = types.ModuleType("antenv.axon_hooks")
    hooks._hook = _ntff_profile_via_ctypes("/opt/axon/libaxon_pjrt.so")
    hooks.get_axon_ntff_profile_hook = lambda: hooks._hook
    hooks.set_axon_ntff_profile_hook = lambda h: setattr(hooks, "_hook", h)
    sys.modules["antenv.axon_hooks"] = hooks
    antenv.axon_hooks = hooks


def kernel(x, W_conv, b_conv):
    global LAST_RESULT
    from concourse import bass_utils

    if TRACE:
        _install_ntff_shim()

    nc = get_built(T)
    in_maps = make_in_maps(x, W_conv, b_conv, T)
    res = bass_utils.run_bass_kernel_spmd(
        nc, in_maps, core_ids=list(range(NCORES)), trace=TRACE, **TRACE_KWARGS
    )
    LAST_RESULT = res
    return assemble(res.results, T)


# revision 23
# speedup vs baseline: 1.0990x; 1.0027x over previous
"""ConvLSTM Trainium2 kernel (Bass/Tile), data-parallel over batch on 8 cores.

Problem shapes (hardcoded): x[16,64,64,256] f32, W_conv[512,192,5], b_conv[512].
reference: per step t, conv1d(concat(x_t, h), W_conv) 'same' pad -> 4 gates ->
LSTM cell update. Returns (outputs[B,T,128,256], h_last, c_last).

Per-core mapping (2 samples/core):
  * conv expressed as PSUM-accumulated matmuls: contraction 960 = 192ch x 5taps,
    done as 8 matmuls of contract-128 per 128-wide output chunk (4 chunks),
    free dim N = 2 samples x 256 = 512 (one full PSUM bank).
  * h-part (128 ch): 5 tap-shifted matmuls against a zero-padded persistent
    h buffer [128, 2, 260]; tap shift = free-dim AP offset (free).
  * x-part (64 ch): two taps packed per matmul: x stored twice in one tile
    (partitions 0-63 unshifted, 64-127 shifted by one column) -> 3 matmuls,
    with tap-paired weight layout prepared host-side.
  * matmul operands in MMDT (bf16 default, fp32r option), fp32 PSUM accumulate.
  * gates: ScalarE activation with fused per-partition bias (sigmoid/tanh are
    in one ACT table set); cell update on VectorE.
"""

import sys

import numpy as np

for _p in ("/opt/trn_rl_repo", "/root/.axon_site/_ro/trn_rl_repo"):
    if _p not in sys.path:
        sys.path.append(_p)

HIDDEN = 128
B, T, C, W = 16, 64, 64, 256
KTAPS = 5
PAD = KTAPS // 2           # 2
WP = W + 2 * PAD           # 260
NCORES = 8
BPC = B // NCORES          # 2 samples per core
NM = 4                     # output-channel chunks of 128 (i, f, o, g)

TRACE = False              # test.py sets True to capture NTFF profile
TRACE_KWARGS = {}
LAST_RESULT = None         # BassKernelResults of the last run (for timing)
MMDT = "bf16"              # matmul operand dtype: "bf16" (FWL) or "f32r"

_BUILT = {}


def _build(t_steps=T, mmdt=None):
    """Build + compile the single-core Bass program (SPMD across cores)."""
    import concourse.bacc as bacc
    import concourse.mybir as mybir
    import concourse.tile as tile

    f32 = mybir.dt.float32
    AF = mybir.ActivationFunctionType
    if mmdt is None:
        mmdt = MMDT
    # matmul operand dtype: float32r = fp32 RNE-rounded to 11 mantissa bits
    # (full PE rate at free dim >= 256, but no fast-weight-load); bfloat16
    # gets FWL so back-to-back matmul spacing is tighter.
    mdt = mybir.dt.bfloat16 if mmdt == "bf16" else mybir.dt.float32r

    nc = bacc.Bacc("TRN2", target_bir_lowering=False, debug=False)

    # x/weights are pre-rounded host-side so their DMAs are plain byte copies.
    xh_d = nc.dram_tensor("xh", [t_steps, C, BPC, W], mdt, kind="ExternalInput")
    wh_d = nc.dram_tensor("wh", [HIDDEN, NM * KTAPS * 128], mdt, kind="ExternalInput")
    wx_d = nc.dram_tensor("wx", [128, NM * 3 * 128], mdt, kind="ExternalInput")
    b_d = nc.dram_tensor("bias", [128, NM], f32, kind="ExternalInput")
    out_d = nc.dram_tensor("out", [t_steps, HIDDEN, BPC, W], mdt, kind="ExternalOutput")
    cout_d = nc.dram_tensor("c_out", [HIDDEN, BPC, W], f32, kind="ExternalOutput")

    xh, wh, wx, bias, out, cout = (
        h.ap() for h in (xh_d, wh_d, wx_d, b_d, out_d, cout_d)
    )

    # matmul emission order of gate chunks: o (chunk 2) last so the
    # sig(o)*tanh(c) tail starts as early as possible.
    MORDER = [0, 1, 3, 2]

    with tile.TileContext(nc) as tc:
        with tc.tile_pool(name="wpool", bufs=1) as wpool, \
             tc.tile_pool(name="spool", bufs=1) as spool, \
             tc.tile_pool(name="gpool", bufs=3) as gpool, \
             tc.tile_pool(name="pspool", bufs=8, space="PSUM") as pspool:

            # --- weights + bias (resident for the whole kernel) ---
            # one big tile per weight group -> one large contiguous DMA each
            wh_sb = wpool.tile([HIDDEN, NM * KTAPS * 128], mdt, name="wh_sb")
            wx_sb = wpool.tile([128, NM * 3 * 128], mdt, name="wx_sb")
            b_sb = wpool.tile([128, NM], f32, name="b_sb")
            nc.sync.dma_start(out=wx_sb, in_=wx)
            nc.sync.dma_start(out=b_sb, in_=bias)
            nc.sync.dma_start(out=wh_sb, in_=wh)
            wh_t = [
                [wh_sb[:, (m * KTAPS + k) * 128:(m * KTAPS + k + 1) * 128]
                 for k in range(KTAPS)]
                for m in range(NM)
            ]
            wx_t = [
                [wx_sb[:, (m * 3 + p) * 128:(m * 3 + p + 1) * 128]
                 for p in range(3)]
                for m in range(NM)
            ]
            b_t = [b_sb[:, m:m + 1] for m in range(NM)]

            # --- persistent state ---
            NXBUF = 3
            NHBUF = 3
            x2 = [spool.tile([128, BPC, WP], mdt, name=f"x2_{i}") for i in range(NXBUF)]
            hp = [spool.tile([HIDDEN, BPC, WP], mdt, name=f"hp_{i}") for i in range(NHBUF)]
            c_st = spool.tile([HIDDEN, BPC, W], f32, name="c_st")
            # pad columns only (interiors are overwritten before every read);
            # VectorE, not GpSimd -- gpsimd's library load delays kernel start.
            for buf in (*x2, *hp):
                nc.vector.memset(buf[:, :, 0:PAD], 0.0)
                nc.vector.memset(buf[:, :, PAD + W - 1:WP], 0.0)
            nc.vector.memset(c_st, 0.0)

            for t in range(t_steps):
                xt = x2[t % NXBUF]
                # top half: x padded by 2; bottom half: same shifted left by 1
                nc.sync.dma_start(out=xt[0:C, :, PAD:PAD + W], in_=xh[t])
                nc.sync.dma_start(out=xt[C:2 * C, :, PAD - 1:PAD - 1 + W], in_=xh[t])
                h_in = hp[t % NHBUF]
                h_out = hp[(t + 1) % NHBUF]

                ps = {
                    m: pspool.tile([128, BPC, W], f32, tag="ps", name=f"ps_{t}_{m}")
                    for m in MORDER
                }
                # x-part first: independent of h, keeps PE busy while the
                # previous step's elementwise tail computes h.
                # t == 0: h is all-zero, skip its matmuls entirely.
                for m in MORDER:
                    for p in range(3):
                        nc.tensor.matmul(
                            ps[m],
                            wx_t[m][p],
                            xt[:, :, 2 * p:2 * p + W],
                            start=(p == 0),
                            stop=(t == 0 and p == 2),
                        )
                if t > 0:
                    for m in MORDER:
                        for k in range(KTAPS):
                            nc.tensor.matmul(
                                ps[m],
                                wh_t[m][k],
                                h_in[:, :, k:k + W],
                                start=False,
                                stop=(k == KTAPS - 1),
                            )

                sig_i = gpool.tile([128, BPC, W], f32, tag="si", name=f"si_{t}")
                sig_f = gpool.tile([128, BPC, W], f32, tag="sf", name=f"sf_{t}")
                tanh_g = gpool.tile([128, BPC, W], f32, tag="tg", name=f"tg_{t}")
                sig_o = gpool.tile([128, BPC, W], f32, tag="so", name=f"so_{t}")
                nc.scalar.activation(out=sig_i, in_=ps[0], func=AF.Sigmoid, bias=b_t[0])
                nc.scalar.activation(out=sig_f, in_=ps[1], func=AF.Sigmoid, bias=b_t[1])
                nc.scalar.activation(out=tanh_g, in_=ps[3], func=AF.Tanh, bias=b_t[3])
                nc.scalar.activation(out=sig_o, in_=ps[2], func=AF.Sigmoid, bias=b_t[2])

                ig = gpool.tile([128, BPC, W], f32, tag="ig", name=f"ig_{t}")
                fc = gpool.tile([128, BPC, W], f32, tag="fc", name=f"fc_{t}")
                nc.vector.tensor_mul(ig, sig_i, tanh_g)
                nc.vector.tensor_mul(fc, sig_f, c_st)
                nc.vector.tensor_add(c_st, ig, fc)
                tnc = gpool.tile([128, BPC, W], f32, tag="tnc", name=f"tnc_{t}")
                nc.scalar.activation(out=tnc, in_=c_st, func=AF.Tanh)
                nc.vector.tensor_mul(h_out[:, :, PAD:PAD + W], sig_o, tnc)
                nc.sync.dma_start(out=out[t], in_=h_out[:, :, PAD:PAD + W])

            nc.sync.dma_start(out=cout, in_=c_st)

    nc.compile()
    return nc


def get_built(t_steps=T, mmdt=None):
    mmdt = MMDT if mmdt is None else mmdt
    key = (t_steps, mmdt)
    if key not in _BUILT:
        _BUILT[key] = _build(t_steps, mmdt)
    return _BUILT[key]


def round_fp32r(a):
    """RNE-round fp32 to the fp32r grid (11 mantissa bits, low 12 bits zero)."""
    u = np.ascontiguousarray(a, np.float32).view(np.uint32)
    u = (u + np.uint32(0x7FF) + ((u >> np.uint32(12)) & np.uint32(1))) \
        & np.uint32(0xFFFFF000)
    return u.view(np.float32)


def _mm_cast(a, mmdt):
    if mmdt == "bf16":
        import ml_dtypes

        return np.asarray(a, np.float32).astype(ml_dtypes.bfloat16)
    return round_fp32r(np.asarray(a, dtype=np.float32))


def prep_weights(W_conv, b_conv, mmdt=None):
    """Host-side weight layout: transposed lhsT tiles + tap-paired x weights,
    packed into single [128, ncols] SBUF images (one DMA each)."""
    mmdt = MMDT if mmdt is None else mmdt
    Wc = np.asarray(W_conv, dtype=np.float32)
    bc = np.asarray(b_conv, dtype=np.float32)
    W4 = Wc.reshape(NM, 128, C + HIDDEN, KTAPS)            # [m, co, ci, k]
    wh4 = W4[:, :, C:, :].transpose(0, 3, 2, 1)            # [m,k,ci(128),co(128)]
    wh = np.ascontiguousarray(
        wh4.reshape(NM * KTAPS, HIDDEN, 128).transpose(1, 0, 2).reshape(
            HIDDEN, NM * KTAPS * 128)
    )
    Wx4 = W4[:, :, :C, :]                                   # [m, co, 64, 5]
    wx4 = np.zeros((NM, 3, 128, 128), np.float32)
    for p in range(2):
        wx4[:, p, 0:C, :] = Wx4[:, :, :, 2 * p].transpose(0, 2, 1)
        wx4[:, p, C:2 * C, :] = Wx4[:, :, :, 2 * p + 1].transpose(0, 2, 1)
    wx4[:, 2, 0:C, :] = Wx4[:, :, :, 4].transpose(0, 2, 1)
    wx = np.ascontiguousarray(
        wx4.reshape(NM * 3, 128, 128).transpose(1, 0, 2).reshape(128, NM * 3 * 128)
    )
    bias = np.ascontiguousarray(bc.reshape(NM, 128).T)      # [128, NM]
    return _mm_cast(wh, mmdt), _mm_cast(wx, mmdt), bias


def make_in_maps(x, W_conv, b_conv, t_steps=T, mmdt=None):
    mmdt = MMDT if mmdt is None else mmdt
    x = _mm_cast(np.asarray(x, dtype=np.float32), mmdt)
    wh, wx, bias = prep_weights(W_conv, b_conv, mmdt)
    in_maps = []
    for cid in range(NCORES):
        xs = x[cid * BPC:(cid + 1) * BPC, :t_steps]         # [2, T', C, W]
        xh = np.ascontiguousarray(xs.transpose(1, 2, 0, 3))  # [T', C, 2, W]
        in_maps.append({"xh": xh, "wh": wh, "wx": wx, "bias": bias})
    return in_maps


def assemble(results, t_steps=T):
    outputs = np.empty((B, t_steps, HIDDEN, W), np.float32)
    c_full = np.empty((B, HIDDEN, W), np.float32)
    for cid in range(NCORES):
        o = np.asarray(results[cid]["out"], np.float32)      # [T', 128, 2, W]
        outputs[cid * BPC:(cid + 1) * BPC] = o.transpose(2, 0, 1, 3)
        c_full[cid * BPC:(cid + 1) * BPC] = results[cid]["c_out"].transpose(1, 0, 2)
    h_full = outputs[:, -1].copy()
    return outputs, h_full, c_full


def _install_ntff_shim():
    """The image's antenv package lacks axon_hooks; recreate it so
    run_bass_kernel_spmd(trace=True) can capture NTFF profiles."""
    import types

    if "antenv.axon_hooks" in sys.modules:
        return
    try:
        import antenv
        from trn_agent_boot.trn_boot import _ntff_profile_via_ctypes
    except ImportError:
        return
    hooks = types.ModuleType("antenv.axon_hooks")
    hooks._hook = _ntff_profile_via_ctypes("/opt/axon/libaxon_pjrt.so")
    hooks.get_axon_ntff_profile_hook = lambda: hooks._hook
    hooks.set_axon_ntff_profile_hook = lambda h: setattr(hooks, "_hook", h)
    sys.modules["antenv.axon_hooks"] = hooks
    antenv.axon_hooks = hooks


def kernel(x, W_conv, b_conv):
    global LAST_RESULT
    from concourse import bass_utils

    if TRACE:
        _install_ntff_shim()

    nc = get_built(T)
    in_maps = make_in_maps(x, W_conv, b_conv, T)
    res = bass_utils.run_bass_kernel_spmd(
        nc, in_maps, core_ids=list(range(NCORES)), trace=TRACE, **TRACE_KWARGS
    )
    LAST_RESULT = res
    return assemble(res.results, T)
